# revision 24
# baseline (speedup 1.0000x reference)
"""Trainium2 Bass kernel for Llama-style GQA attention (B=1, S=2048, D=4096,
32 q heads / 8 kv heads, head_dim 128, neox RoPE, causal).

Sharding: tensor-parallel over kv heads across 8 NeuronCores. Core c owns
kv head c and q heads [4c, 4c+4): Wq cols [512c, 512c+512), Wk/Wv cols
[128c, 128c+128), Wo rows [512c, 512c+512). Each core computes a full
[D, S] partial of the output (o_proj row-parallel); host sums the 8 partials.

All matmul data is fp16 (PSUM accumulation f32); inputs are cast on the host.
Per-core kernel:
  A) Per s-slice of 512 (two DMA half-slices of 256): project q0..q3/k
     transposed ([dh, s], weights stationary, hiddenT moving) and v in
     natural [s, dh] layout (hiddenT-block stationary, Wv moving; four
     128-row groups packed per PSUM bank). RoPE is fused per-slice: ACT
     drains PSUM->fp16, a rotate-half PE matmul, then DVE fp16 combines
     x*cos + rot(x)*sin. PE never idles between slices.
  B) Attention per q-slice of 512, interleaved with the previous q-slice's
     o_proj matmuls as PE filler: scoresT pairs two heads in one
     [128,1024] PSUM tile -> one Exp per pair (bias=-4 keeps exp in fp16
     range; softmax is shift-invariant), diagonal tiles width-trimmed with
     a [128,128] triangular mask add, ex in fp16. pv accumulates O^T in
     paired PSUM. Softmax denominators: DVE fp16 accumulate of ex tiles,
     then a ones-stationary matmul replicates the partition sum; normalize
     happens in the PSUM->SBUF drain mul.
  C) o_proj: out^T[D,S] partial, Wo stationary, O^T moving, PSUM pairs
     (two 128-row D blocks) -> fp16 SBUF -> DRAM.
"""

import threading
from dataclasses import dataclass

import numpy as np


@dataclass(frozen=True)
class Cfg:
    S: int = 2048
    D: int = 4096
    HQ: int = 4        # q heads per core
    DH: int = 128
    QSL: int = 512     # q-slice width for attention
    theta: float = 10000.0
    cores: int = 8


FULL = Cfg()
NEG = -1.0e9
CBIAS = -6.0  # exp(scale*score + CBIAS): keeps exp sums in fp16 range


def build_nc(cfg: Cfg):
    import concourse.bass as bass  # noqa: F401
    import concourse.mybir as mybir
    import concourse.tile as tile
    from concourse import bacc
    from concourse.masks import make_identity  # noqa: F401

    F16 = mybir.dt.float16
    F32 = mybir.dt.float32
    Exp = mybir.ActivationFunctionType.Exp

    S, D, HQ, DH, QSL = cfg.S, cfg.D, cfg.HQ, cfg.DH, cfg.QSL
    DT = D // 128            # contraction d-tiles
    NSL = S // 512           # 512-wide s-slices
    NKT = S // 128           # k-position tiles
    NQS = S // QSL           # q slices
    NDP = DT // 2            # o_proj D-block pairs
    scale = float(DH) ** -0.5

    nc = bacc.Bacc("TRN2", target_bir_lowering=False, debug=False,
                   num_devices=cfg.cores)

    ht = nc.dram_tensor("ht", [2 * NSL, 128, DT * 256], F16,
                        kind="ExternalInput").ap()
    wq = nc.dram_tensor("wq", [128, DT, HQ * DH], F16,
                        kind="ExternalInput").ap()
    wk = nc.dram_tensor("wk", [128, DT, DH], F16, kind="ExternalInput").ap()
    wv = nc.dram_tensor("wv", [128, DT, DH], F16, kind="ExternalInput").ap()
    wo = nc.dram_tensor("wo", [128, HQ, D], F16, kind="ExternalInput").ap()
    cosh = nc.dram_tensor("cosh", [128, S], F16, kind="ExternalInput").ap()
    sinh = nc.dram_tensor("sinh", [128, S], F16, kind="ExternalInput").ap()
    mneg = nc.dram_tensor("mneg", [128, 128], F32, kind="ExternalInput").ap()
    rotm = nc.dram_tensor("rotm", [128, 256], F16, kind="ExternalInput").ap()
    outT = nc.dram_tensor("outT", [D, S], F16, kind="ExternalOutput").ap()

    with tile.TileContext(nc) as tc, \
            tc.tile_pool(name="persist", bufs=1) as pp:
        qT = [pp.tile([128, S], F16, tag=f"qT{g}", name=f"qT{g}")
              for g in range(HQ)]
        kT = pp.tile([128, S], F16, tag="kT")
        v_all = pp.tile([128, NKT, DH], F16, tag="vall")
        o_attn = [pp.tile([128, S], F16, tag=f"oT{g}", name=f"oT{g}")
                  for g in range(HQ)]
        cos_sb = pp.tile([128, S], F16, tag="cos")
        sin_sb = pp.tile([128, S], F16, tag="sin")
        mneg_sb = pp.tile([128, 128], F32, tag="mneg")
        rotm_sb = pp.tile([128, 256], F16, tag="rotm")
        wo_sb = pp.tile([128, HQ, D], F16, tag="wo")
        rot_sb = rotm_sb[:, 0:128]
        ones_sb = rotm_sb[:, 128:256]

        cb_sb = pp.tile([128, 1], F32, tag="cbias")
        nc.gpsimd.memset(cb_sb, CBIAS)
        nc.sync.dma_start(out=mneg_sb, in_=mneg)
        nc.sync.dma_start(out=rotm_sb, in_=rotm)

        # ---------------- phase A: projections + RoPE ----------------
        with (
            tc.tile_pool(name="wpool", bufs=1) as wp,
            tc.tile_pool(name="htp", bufs=3) as hp,
            tc.tile_pool(name="psA", bufs=6, space="PSUM") as psA,
            tc.tile_pool(name="trA", bufs=2, space="PSUM") as trA,
            tc.tile_pool(name="rawp", bufs=5) as rp,
            tc.tile_pool(name="prhp", bufs=2) as php,
            tc.tile_pool(name="rtmp", bufs=4) as rtp,
        ):
            wk_sb = wp.tile([128, DT, DH], F16, tag="wk")
            wv_sb = wp.tile([128, DT, DH], F16, tag="wv")
            wq_sb = wp.tile([128, DT, HQ * DH], F16, tag="wq")

            # DMA order: wk/ht0 in fine chunks so the k-pass starts almost
            # immediately, then wq in d-chunks, wv, tables, wo prefetch.
            ht_tiles = {}

            def ht_dma(hs, chunks=1):
                t = hp.tile([128, DT * 256], F16, tag="ht", name=f"ht{hs}")
                cw = DT * 256 // chunks
                for ci in range(chunks):
                    nc.sync.dma_start(out=t[:, ci * cw:(ci + 1) * cw],
                                      in_=ht[hs][:, ci * cw:(ci + 1) * cw])
                ht_tiles[hs] = t

            nc.sync.dma_start(out=wk_sb[:, 0:16, :], in_=wk[:, 0:16, :])
            ht_dma(0, chunks=4)
            nc.sync.dma_start(out=wk_sb[:, 16:32, :], in_=wk[:, 16:32, :])
            nc.sync.dma_start(out=wq_sb[:, 0:8, :], in_=wq[:, 0:8, :])
            nc.sync.dma_start(out=wq_sb[:, 8:16, :], in_=wq[:, 8:16, :])
            nc.sync.dma_start(out=wv_sb, in_=wv)
            ht_dma(1)
            nc.sync.dma_start(out=wq_sb[:, 16:24, :], in_=wq[:, 16:24, :])
            nc.sync.dma_start(out=wq_sb[:, 24:32, :], in_=wq[:, 24:32, :])
            nc.sync.dma_start(out=cos_sb, in_=cosh)
            nc.sync.dma_start(out=sin_sb, in_=sinh)
            ht_dma(2)
            nc.sync.dma_start(out=wo_sb, in_=wo)

            for sl in range(NSL):
                ssl = slice(sl * 512, (sl + 1) * 512)
                psQ = [psA.tile([128, 512], F32, tag="psA", name=f"psQ{g}")
                       for g in range(HQ)]
                psK = psA.tile([128, 512], F32, tag="psA", name="psK")
                psV = psA.tile([128, 512], F32, tag="psA", name="psV")
                def kpass(ht_t, hcols):
                    for d in range(DT):
                        nc.tensor.matmul(
                            psK[:, hcols], wk_sb[:, d, :],
                            ht_t[:, d * 256:(d + 1) * 256],
                            start=(d == 0), stop=(d == DT - 1))

                def qpass(ht_t, hcols):
                    for d in range(DT):
                        mv = ht_t[:, d * 256:(d + 1) * 256]
                        for g in range(HQ):
                            nc.tensor.matmul(
                                psQ[g][:, hcols],
                                wq_sb[:, d, g * DH:(g + 1) * DH], mv,
                                start=(d == 0), stop=(d == DT - 1))

                def vpass(ht_t, h):
                    # natural layout, hiddenT block stationary
                    for sb in range(2):
                        vc = slice((2 * h + sb) * 128, (2 * h + sb + 1) * 128)
                        for d in range(DT):
                            nc.tensor.matmul(
                                psV[:, vc],
                                ht_t[:, d * 256 + sb * 128:
                                     d * 256 + (sb + 1) * 128],
                                wv_sb[:, d, :],
                                start=(d == 0), stop=(d == DT - 1))

                def rope(raw, dstT):
                    pr = trA.tile([128, 512], F32, tag="pr", name="pr")
                    nc.tensor.matmul(pr, rot_sb, raw, start=True, stop=True)
                    prh = php.tile([128, 512], F16, tag="prh")
                    nc.scalar.copy(prh, pr)
                    rs = rtp.tile([128, 512], F16, tag="rs")
                    nc.vector.tensor_mul(rs, prh, sin_sb[:, ssl])
                    cc = rtp.tile([128, 512], F16, tag="cc")
                    nc.vector.tensor_mul(cc, raw, cos_sb[:, ssl])
                    nc.vector.tensor_add(dstT[:, ssl], cc, rs)

                def ht_get(h):
                    hs = 2 * sl + h
                    if hs + 2 < 2 * NSL and (hs + 2) not in ht_tiles:
                        ht_dma(hs + 2)
                    return ht_tiles.pop(hs)

                # h0: k first (its weights arrive first); h1: q first with
                # eager drains so the PSUM accumulators release while the
                # k/v passes still run on the PE (faster A->C transition)
                ht0 = ht_get(0)
                kpass(ht0, slice(0, 256))
                qpass(ht0, slice(0, 256))
                vpass(ht0, 0)
                ht1 = ht_get(1)
                qpass(ht1, slice(256, 512))
                qraws = []
                for g in range(HQ):
                    qr = rp.tile([128, 512], F16, tag="raw", name=f"qraw{g}")
                    nc.scalar.copy(qr, psQ[g])
                    qraws.append(qr)
                kpass(ht1, slice(256, 512))
                for g in range(HQ):
                    rope(qraws[g], qT[g])
                kr = rp.tile([128, 512], F16, tag="raw", name="kraw")
                nc.scalar.copy(kr, psK)
                vpass(ht1, 1)
                rope(kr, kT)
                nc.scalar.copy(v_all[:, 4 * sl:4 * sl + 4, :], psV)

        # ---------------- phase B+C: attention + o_proj ----------------
        outT_r = outT.rearrange("(x p) s -> p x s", p=128)
        with (
            tc.tile_pool(name="psO", bufs=2, space="PSUM") as psO,
            tc.tile_pool(name="trS", bufs=2, space="PSUM") as trS,
            tc.tile_pool(name="expp", bufs=6) as ep,
            tc.tile_pool(name="accp", bufs=4) as ap_,
            tc.tile_pool(name="dnp", bufs=4) as dp,
            tc.tile_pool(name="ocp", bufs=4) as ocp,
        ):
            def emit_pf_pair(qs, dpair):
                qsl = slice(qs * QSL, (qs + 1) * QSL)
                pf = trS.tile([128, 1024], F32, tag="trS", name="pf")
                for g in range(HQ):
                    for i in range(2):
                        nc.tensor.matmul(
                            pf[:, i * 512:(i + 1) * 512],
                            wo_sb[:, g, (2 * dpair + i) * 128:
                                  (2 * dpair + i + 1) * 128],
                            o_attn[g][:, qsl],
                            start=(g == 0), stop=(g == HQ - 1))
                oc = ocp.tile([128, 1024], F16, tag="oc")
                # parallel half-casts (ACT + DVE) release the PSUM slot fast
                nc.scalar.copy(oc[:, 0:512], pf[:, 0:512])
                nc.vector.tensor_copy(oc[:, 512:1024], pf[:, 512:1024])
                nc.sync.dma_start(
                    out=outT_r[:, 2 * dpair:2 * dpair + 2, qsl], in_=oc)

            pend_pv = None  # pv of the previous tile, emitted one tile late

            def emit_pv(pv):
                po, kind, exd, t, dd, nkt = pv
                for p in range(2):
                    for i in range(2):
                        h = slice(i * 512 + dd, (i + 1) * 512)
                        if kind == "pair":
                            mv = exd[p][:, h]
                        else:  # packed: 4 heads side by side, width w each
                            ex, w = exd
                            g = 2 * p + i
                            mv = ex[:, g * w:(g + 1) * w]
                        nc.tensor.matmul(po[p][:, h], v_all[:, t, :], mv,
                                         start=(t == 0),
                                         stop=(t == nkt - 1))

            for qs in range(NQS):
                qsl = slice(qs * QSL, (qs + 1) * QSL)
                nkt = 4 * (qs + 1)
                po = [psO.tile([128, 1024], F32, tag="po", name=f"po{p}")
                      for p in range(2)]
                acc = [ap_.tile([128, 1024], F16, tag="acc", name=f"acc{p}")
                       for p in range(2)]
                # o_proj filler work from the previous q-slice, spread over
                # t=1..nkt-1 (fillers at t=0 would wait on the fresh drain)
                pending = list(range(NDP)) if qs > 0 else []
                reserve = 2 if pending else 0
                navail = len(pending) - reserve
                # fillers weighted 3x toward diagonal tiles (narrow scores =
                # little PE cover there while the exp chains are longest)
                wts = [0] + [3 if (t * 128 >= qs * QSL) else 1
                             for t in range(1, nkt)]
                wtot = sum(wts) or 1
                cum = 0
                fills = []
                for t in range(nkt):
                    cum += wts[t]
                    fills.append((cum * navail) // wtot)

                for t in range(nkt):
                    n_fill = 0
                    if pending and t > 0:
                        n_fill = fills[t] - fills[t - 1]
                    delta = t * 128 - qs * QSL
                    dd = max(delta, 0)
                    ktile = kT[:, t * 128:(t + 1) * 128]
                    qslc = slice(qs * QSL + dd, (qs + 1) * QSL)
                    if dd >= 256:
                        # narrow diagonal tile: pack all 4 heads' trimmed
                        # scores in ONE slot (halves ring pressure, one exp)
                        w = 512 - dd
                        ps = trS.tile([128, 1024], F32, tag="trS", name="psD")
                        for g in range(HQ):
                            nc.tensor.matmul(ps[:, g * w:(g + 1) * w],
                                             ktile, qT[g][:, qslc],
                                             start=True, stop=True)
                        for g in range(HQ):
                            blk = slice(g * w, g * w + 128)
                            nc.vector.tensor_add(ps[:, blk], ps[:, blk],
                                                 mneg_sb)
                        ex = ep.tile([128, 1024], F16, tag="ex")
                        nc.scalar.activation(ex[:, 0:4 * w], ps[:, 0:4 * w],
                                             Exp, bias=cb_sb, scale=scale)
                        this_pv = (po, "packed", (ex, w), t, dd, nkt)
                    else:
                        exs = []
                        for p in range(2):
                            ps = trS.tile([128, 1024], F32, tag="trS",
                                          name="psS")
                            for i in range(2):
                                g = 2 * p + i
                                nc.tensor.matmul(
                                    ps[:, i * 512 + dd:(i + 1) * 512],
                                    ktile, qT[g][:, qslc],
                                    start=True, stop=True)
                            if delta >= 0:
                                for i in range(2):
                                    blk = slice(i * 512 + dd,
                                                i * 512 + dd + 128)
                                    nc.vector.tensor_add(ps[:, blk],
                                                         ps[:, blk], mneg_sb)
                            ex = ep.tile([128, 1024], F16, tag="ex")
                            # dd=128: full-pair exp is as cheap as two trimmed
                            # ops; the stale strips are never read (pv and the
                            # denominator adds are trimmed to [dd:512])
                            nc.scalar.activation(ex, ps, Exp,
                                                 bias=cb_sb, scale=scale)
                            exs.append(ex)
                        this_pv = (po, "pair", exs, t, dd, nkt)
                    # previous tile's pv: independent PE work that covers the
                    # exp latency and the score-slot WAR on the in-order PE
                    if pend_pv is not None:
                        emit_pv(pend_pv)
                    for _ in range(n_fill):
                        if len(pending) > reserve:
                            emit_pf_pair(qs - 1, pending.pop(0))
                    pend_pv = this_pv
                    for p in range(2):
                        if t == 0:
                            nc.vector.tensor_copy(acc[p], exs[p])
                        elif dd == 0:
                            nc.vector.tensor_add(acc[p], acc[p], exs[p])
                        elif dd >= 256:
                            for i in range(2):
                                h = slice(i * 512 + dd, (i + 1) * 512)
                                g = 2 * p + i
                                nc.vector.tensor_add(acc[p][:, h],
                                                     acc[p][:, h],
                                                     ex[:, g * w:(g + 1) * w])
                        else:
                            for i in range(2):
                                h = slice(i * 512 + dd, (i + 1) * 512)
                                nc.vector.tensor_add(acc[p][:, h],
                                                     acc[p][:, h],
                                                     exs[p][:, h])

                # denominators (need only the DVE acc chain, not the pvs)
                dns = []
                pds = []
                for p in range(2):
                    pd = trS.tile([128, 1024], F32, tag="trS", name="pd")
                    for i in range(2):
                        h = slice(i * 512, (i + 1) * 512)
                        nc.tensor.matmul(pd[:, h], ones_sb, acc[p][:, h],
                                         start=True, stop=True)
                    pds.append(pd)
                # last tile's pv before the reciprocal chain needs it
                emit_pv(pend_pv)
                pend_pv = None
                for p in range(2):
                    # fast SBUF copy releases the PSUM slot before the
                    # (slower) reciprocal runs
                    pdc = dp.tile([128, 1024], F32, tag="pdc")
                    nc.vector.tensor_copy(pdc, pds[p])
                    dn = dp.tile([128, 1024], F32, tag="dn")
                    nc.vector.reciprocal_approx_fast(out=dn, in_=pdc)
                    dns.append(dn)
                for p in range(2):
                    for i in range(2):
                        g = 2 * p + i
                        h = slice(i * 512, (i + 1) * 512)
                        nc.vector.tensor_mul(o_attn[g][:, qsl],
                                             po[p][:, h], dns[p][:, h])
                # reserved PE filler covers the reciprocal+drain latency
                while pending:
                    emit_pf_pair(qs - 1, pending.pop(0))

            for dpair in range(NDP):
                emit_pf_pair(NQS - 1, dpair)

    nc.compile()
    return nc


def make_tables(cfg: Cfg, position_ids: np.ndarray):
    """cos/sin [128, S] fp16: row d holds cos/sin(pos * invfreq[d % 64])."""
    half = cfg.DH // 2
    inv = 1.0 / (cfg.theta ** (np.arange(half, dtype=np.float64) * 2.0 / cfg.DH))
    pos = np.asarray(position_ids).reshape(-1).astype(np.float64)
    ang = inv[:, None] * pos[None, :]
    cosT = np.concatenate([np.cos(ang), np.cos(ang)], 0).astype(np.float16)
    sinT = np.concatenate([np.sin(ang), np.sin(ang)], 0).astype(np.float16)
    return cosT, sinT


def make_rotm(cfg: Cfg):
    """[128, 256] fp16: cols 0-127 rotate-half matrix (out = M^T @ x ->
    out[:64] = -x[64:], out[64:] = x[:64]); cols 128-255 all ones."""
    half = cfg.DH // 2
    m = np.zeros((128, 256), np.float16)
    for i in range(half):
        m[i + half, i] = -1.0
        m[i, i + half] = 1.0
    m[:, 128:] = 1.0
    return m


def make_mneg():
    """[128, 128] f32 strict lower-triangular NEG: mask[p, j] = NEG if j < p."""
    j = np.arange(128)[None, :]
    p = np.arange(128)[:, None]
    return np.where(j < p, NEG, 0.0).astype(np.float32)


_cache = threading.Lock()
_nc_full = None


def _get_nc():
    global _nc_full
    with _cache:
        if _nc_full is None:
            _nc_full = build_nc(FULL)
    return _nc_full


def _prep_shared(cfg: Cfg, position_ids, hidden_states):
    S, D, DT = cfg.S, cfg.D, cfg.D // 128
    h16 = np.asarray(hidden_states, np.float32).reshape(S, D).T.astype(np.float16)
    # [D, S] -> [2*NSL half-slices, 128 partitions, DT*256]
    ht = np.ascontiguousarray(
        h16.reshape(DT, 128, 2 * (S // 512), 256).transpose(2, 1, 0, 3)
        .reshape(2 * (S // 512), 128, DT * 256))
    cosT, sinT = make_tables(cfg, position_ids)
    return ht, cosT, sinT


def core_inputs(cfg: Cfg, c: int, shared, Wq, Wk, Wv, Wo):
    S, D, HQ, DH = cfg.S, cfg.D, cfg.HQ, cfg.DH
    DT = D // 128
    ht, cosT, sinT = shared
    qc = slice(c * HQ * DH, (c + 1) * HQ * DH)
    kc = slice(c * DH, (c + 1) * DH)

    def wtile(w, cols, n):
        w16 = np.asarray(w, np.float32)[:, cols].astype(np.float16)
        return np.ascontiguousarray(w16.reshape(DT, 128, n).transpose(1, 0, 2))

    wo16 = np.asarray(Wo, np.float32)[qc, :].astype(np.float16)
    wo_t = np.ascontiguousarray(wo16.reshape(HQ, 128, D).transpose(1, 0, 2))
    return {
        "ht": ht,
        "wq": wtile(Wq, qc, HQ * DH),
        "wk": wtile(Wk, kc, DH),
        "wv": wtile(Wv, kc, DH),
        "wo": wo_t,
        "cosh": cosT,
        "sinh": sinT,
        "mneg": make_mneg(),
        "rotm": make_rotm(cfg),
    }


def kernel(position_ids, hidden_states, Wq, Wk, Wv, Wo, _trace=False):
    from concourse.bass_utils import run_bass_kernel_spmd

    cfg = FULL
    nc = _get_nc()
    shared = _prep_shared(cfg, position_ids, hidden_states)
    in_maps = [core_inputs(cfg, c, shared, Wq, Wk, Wv, Wo)
               for c in range(cfg.cores)]
    res = run_bass_kernel_spmd(nc, in_maps, core_ids=list(range(cfg.cores)),
                               trace=_trace)
    out = np.zeros((cfg.S, cfg.D), np.float64)
    for c in range(cfg.cores):
        out += res.results[c]["outT"].T.astype(np.float64)
    ret = out.astype(np.float32).reshape(1, cfg.S, cfg.D)
    if _trace:
        return ret, res
    return ret


# revision 27
# speedup vs baseline: 1.0057x; 1.0057x over previous
"""Trainium2 Bass kernel for Llama-style GQA attention (B=1, S=2048, D=4096,
32 q heads / 8 kv heads, head_dim 128, neox RoPE, causal).

Sharding: tensor-parallel over kv heads across 8 NeuronCores. Core c owns
kv head c and q heads [4c, 4c+4): Wq cols [512c, 512c+512), Wk/Wv cols
[128c, 128c+128), Wo rows [512c, 512c+512). Each core computes a full
[D, S] partial of the output (o_proj row-parallel); host sums the 8 partials.

All matmul data is fp16 (PSUM accumulation f32); inputs are cast on the host.
Per-core kernel:
  A) Per s-slice of 512 (two DMA half-slices of 256): project q0..q3/k
     transposed ([dh, s], weights stationary, hiddenT moving) and v in
     natural [s, dh] layout (hiddenT-block stationary, Wv moving; four
     128-row groups packed per PSUM bank). RoPE is fused per-slice: ACT
     drains PSUM->fp16, a rotate-half PE matmul, then DVE fp16 combines
     x*cos + rot(x)*sin. PE never idles between slices.
  B) Attention per q-slice of 512, interleaved with the previous q-slice's
     o_proj matmuls as PE filler: scoresT pairs two heads in one
     [128,1024] PSUM tile -> one Exp per pair (bias=-4 keeps exp in fp16
     range; softmax is shift-invariant), diagonal tiles width-trimmed with
     a [128,128] triangular mask add, ex in fp16. pv accumulates O^T in
     paired PSUM. Softmax denominators: DVE fp16 accumulate of ex tiles,
     then a ones-stationary matmul replicates the partition sum; normalize
     happens in the PSUM->SBUF drain mul.
  C) o_proj: out^T[D,S] partial, Wo stationary, O^T moving, PSUM pairs
     (two 128-row D blocks) -> fp16 SBUF -> DRAM.
"""

import threading
from dataclasses import dataclass

import numpy as np


@dataclass(frozen=True)
class Cfg:
    S: int = 2048
    D: int = 4096
    HQ: int = 4        # q heads per core
    DH: int = 128
    QSL: int = 512     # q-slice width for attention
    theta: float = 10000.0
    cores: int = 8


FULL = Cfg()
NEG = -1.0e9
CBIAS = -6.0  # exp(scale*score + CBIAS): keeps exp sums in fp16 range


def build_nc(cfg: Cfg):
    import concourse.bass as bass  # noqa: F401
    import concourse.mybir as mybir
    import concourse.tile as tile
    from concourse import bacc
    from concourse.masks import make_identity  # noqa: F401

    F16 = mybir.dt.float16
    F32 = mybir.dt.float32
    Exp = mybir.ActivationFunctionType.Exp

    S, D, HQ, DH, QSL = cfg.S, cfg.D, cfg.HQ, cfg.DH, cfg.QSL
    DT = D // 128            # contraction d-tiles
    NSL = S // 512           # 512-wide s-slices
    NKT = S // 128           # k-position tiles
    NQS = S // QSL           # q slices
    NDP = DT // 2            # o_proj D-block pairs
    scale = float(DH) ** -0.5

    nc = bacc.Bacc("TRN2", target_bir_lowering=False, debug=False,
                   num_devices=cfg.cores)

    ht = nc.dram_tensor("ht", [2 * NSL, 128, DT * 256], F16,
                        kind="ExternalInput").ap()
    wq = nc.dram_tensor("wq", [128, DT, HQ * DH], F16,
                        kind="ExternalInput").ap()
    wk = nc.dram_tensor("wk", [128, DT, DH], F16, kind="ExternalInput").ap()
    wv = nc.dram_tensor("wv", [128, DT, DH], F16, kind="ExternalInput").ap()
    wo = nc.dram_tensor("wo", [128, HQ, D], F16, kind="ExternalInput").ap()
    cosh = nc.dram_tensor("cosh", [128, S], F16, kind="ExternalInput").ap()
    sinh = nc.dram_tensor("sinh", [128, S], F16, kind="ExternalInput").ap()
    mneg = nc.dram_tensor("mneg", [128, 128], F32, kind="ExternalInput").ap()
    rotm = nc.dram_tensor("rotm", [128, 256], F16, kind="ExternalInput").ap()
    outT = nc.dram_tensor("outT", [D, S], F16, kind="ExternalOutput").ap()

    with tile.TileContext(nc) as tc, \
            tc.tile_pool(name="persist", bufs=1) as pp:
        qT = [pp.tile([128, S], F16, tag=f"qT{g}", name=f"qT{g}")
              for g in range(HQ)]
        kT = pp.tile([128, S], F16, tag="kT")
        v_all = pp.tile([128, NKT, DH], F16, tag="vall")
        o_attn = [pp.tile([128, S], F16, tag=f"oT{g}", name=f"oT{g}")
                  for g in range(HQ)]
        cos_sb = pp.tile([128, S], F16, tag="cos")
        sin_sb = pp.tile([128, S], F16, tag="sin")
        mneg_sb = pp.tile([128, 128], F32, tag="mneg")
        rotm_sb = pp.tile([128, 256], F16, tag="rotm")
        wo_sb = pp.tile([128, HQ, D], F16, tag="wo")
        rot_sb = rotm_sb[:, 0:128]
        ones_sb = rotm_sb[:, 128:256]

        cb_sb = pp.tile([128, 1], F32, tag="cbias")
        nc.gpsimd.memset(cb_sb, CBIAS)
        nc.sync.dma_start(out=mneg_sb, in_=mneg)
        nc.sync.dma_start(out=rotm_sb, in_=rotm)

        # ---------------- phase A: projections + RoPE ----------------
        with (
            tc.tile_pool(name="wpool", bufs=1) as wp,
            tc.tile_pool(name="htp", bufs=3) as hp,
            tc.tile_pool(name="psA", bufs=6, space="PSUM") as psA,
            tc.tile_pool(name="trA", bufs=2, space="PSUM") as trA,
            tc.tile_pool(name="rawp", bufs=5) as rp,
            tc.tile_pool(name="prhp", bufs=2) as php,
            tc.tile_pool(name="rtmp", bufs=4) as rtp,
        ):
            wk_sb = wp.tile([128, DT, DH], F16, tag="wk")
            wv_sb = wp.tile([128, DT, DH], F16, tag="wv")
            wq_sb = wp.tile([128, DT, HQ * DH], F16, tag="wq")

            # DMA order: wk/ht0 in fine chunks so the k-pass starts almost
            # immediately, then wq in d-chunks, wv, tables, wo prefetch.
            ht_tiles = {}

            def ht_dma(hs, chunks=1):
                t = hp.tile([128, DT * 256], F16, tag="ht", name=f"ht{hs}")
                cw = DT * 256 // chunks
                for ci in range(chunks):
                    nc.sync.dma_start(out=t[:, ci * cw:(ci + 1) * cw],
                                      in_=ht[hs][:, ci * cw:(ci + 1) * cw])
                ht_tiles[hs] = t

            nc.sync.dma_start(out=wk_sb[:, 0:8, :], in_=wk[:, 0:8, :])
            ht_dma(0, chunks=8)
            nc.sync.dma_start(out=wk_sb[:, 8:32, :], in_=wk[:, 8:32, :])
            nc.sync.dma_start(out=wq_sb[:, 0:8, :], in_=wq[:, 0:8, :])
            nc.sync.dma_start(out=wq_sb[:, 8:16, :], in_=wq[:, 8:16, :])
            nc.sync.dma_start(out=wv_sb, in_=wv)
            ht_dma(1)
            nc.sync.dma_start(out=wq_sb[:, 16:24, :], in_=wq[:, 16:24, :])
            nc.sync.dma_start(out=wq_sb[:, 24:32, :], in_=wq[:, 24:32, :])
            nc.sync.dma_start(out=cos_sb, in_=cosh)
            nc.sync.dma_start(out=sin_sb, in_=sinh)
            ht_dma(2)
            nc.sync.dma_start(out=wo_sb, in_=wo)

            for sl in range(NSL):
                ssl = slice(sl * 512, (sl + 1) * 512)
                psQ = [psA.tile([128, 512], F32, tag="psA", name=f"psQ{g}")
                       for g in range(HQ)]
                psK = psA.tile([128, 512], F32, tag="psA", name="psK")
                psV = psA.tile([128, 512], F32, tag="psA", name="psV")
                def kpass(ht_t, hcols):
                    for d in range(DT):
                        nc.tensor.matmul(
                            psK[:, hcols], wk_sb[:, d, :],
                            ht_t[:, d * 256:(d + 1) * 256],
                            start=(d == 0), stop=(d == DT - 1))

                def qpass(ht_t, hcols):
                    for d in range(DT):
                        mv = ht_t[:, d * 256:(d + 1) * 256]
                        for g in range(HQ):
                            nc.tensor.matmul(
                                psQ[g][:, hcols],
                                wq_sb[:, d, g * DH:(g + 1) * DH], mv,
                                start=(d == 0), stop=(d == DT - 1))

                def vpass(ht_t, h):
                    # natural layout, hiddenT block stationary
                    for sb in range(2):
                        vc = slice((2 * h + sb) * 128, (2 * h + sb + 1) * 128)
                        for d in range(DT):
                            nc.tensor.matmul(
                                psV[:, vc],
                                ht_t[:, d * 256 + sb * 128:
                                     d * 256 + (sb + 1) * 128],
                                wv_sb[:, d, :],
                                start=(d == 0), stop=(d == DT - 1))

                def rope(raw, dstT):
                    pr = trA.tile([128, 512], F32, tag="pr", name="pr")
                    nc.tensor.matmul(pr, rot_sb, raw, start=True, stop=True)
                    prh = php.tile([128, 512], F16, tag="prh")
                    nc.scalar.copy(prh, pr)
                    rs = rtp.tile([128, 512], F16, tag="rs")
                    nc.vector.tensor_mul(rs, prh, sin_sb[:, ssl])
                    cc = rtp.tile([128, 512], F16, tag="cc")
                    nc.vector.tensor_mul(cc, raw, cos_sb[:, ssl])
                    nc.vector.tensor_add(dstT[:, ssl], cc, rs)

                def ht_get(h):
                    hs = 2 * sl + h
                    if hs + 2 < 2 * NSL and (hs + 2) not in ht_tiles:
                        ht_dma(hs + 2)
                    return ht_tiles.pop(hs)

                # h0: k first (its weights arrive first); h1: q first with
                # eager drains so the PSUM accumulators release while the
                # k/v passes still run on the PE (faster A->C transition)
                ht0 = ht_get(0)
                kpass(ht0, slice(0, 256))
                qpass(ht0, slice(0, 256))
                vpass(ht0, 0)
                ht1 = ht_get(1)
                qpass(ht1, slice(256, 512))
                qraws = []
                for g in range(HQ):
                    qr = rp.tile([128, 512], F16, tag="raw", name=f"qraw{g}")
                    nc.scalar.copy(qr, psQ[g])
                    qraws.append(qr)
                kpass(ht1, slice(256, 512))
                for g in range(HQ):
                    rope(qraws[g], qT[g])
                kr = rp.tile([128, 512], F16, tag="raw", name="kraw")
                nc.scalar.copy(kr, psK)
                vpass(ht1, 1)
                rope(kr, kT)
                nc.scalar.copy(v_all[:, 4 * sl:4 * sl + 4, :], psV)

        # ---------------- phase B+C: attention + o_proj ----------------
        outT_r = outT.rearrange("(x p) s -> p x s", p=128)
        with (
            tc.tile_pool(name="psO", bufs=2, space="PSUM") as psO,
            tc.tile_pool(name="trS", bufs=2, space="PSUM") as trS,
            tc.tile_pool(name="expp", bufs=6) as ep,
            tc.tile_pool(name="accp", bufs=4) as ap_,
            tc.tile_pool(name="dnp", bufs=4) as dp,
            tc.tile_pool(name="ocp", bufs=4) as ocp,
        ):
            def emit_pf_pair(qs, dpair):
                qsl = slice(qs * QSL, (qs + 1) * QSL)
                pf = trS.tile([128, 1024], F32, tag="trS", name="pf")
                for g in range(HQ):
                    for i in range(2):
                        nc.tensor.matmul(
                            pf[:, i * 512:(i + 1) * 512],
                            wo_sb[:, g, (2 * dpair + i) * 128:
                                  (2 * dpair + i + 1) * 128],
                            o_attn[g][:, qsl],
                            start=(g == 0), stop=(g == HQ - 1))
                oc = ocp.tile([128, 1024], F16, tag="oc")
                # parallel half-casts (ACT + DVE) release the PSUM slot fast
                nc.scalar.copy(oc[:, 0:512], pf[:, 0:512])
                nc.vector.tensor_copy(oc[:, 512:1024], pf[:, 512:1024])
                nc.sync.dma_start(
                    out=outT_r[:, 2 * dpair:2 * dpair + 2, qsl], in_=oc)

            pend_pv = None  # pv of the previous tile, emitted one tile late

            def emit_pv(pv):
                po, kind, exd, t, dd, nkt = pv
                for p in range(2):
                    for i in range(2):
                        h = slice(i * 512 + dd, (i + 1) * 512)
                        if kind == "pair":
                            mv = exd[p][:, h]
                        else:  # packed: 4 heads side by side, width w each
                            ex, w = exd
                            g = 2 * p + i
                            mv = ex[:, g * w:(g + 1) * w]
                        nc.tensor.matmul(po[p][:, h], v_all[:, t, :], mv,
                                         start=(t == 0),
                                         stop=(t == nkt - 1))

            for qs in range(NQS):
                qsl = slice(qs * QSL, (qs + 1) * QSL)
                nkt = 4 * (qs + 1)
                po = [psO.tile([128, 1024], F32, tag="po", name=f"po{p}")
                      for p in range(2)]
                acc = [ap_.tile([128, 1024], F16, tag="acc", name=f"acc{p}")
                       for p in range(2)]
                # o_proj filler work from the previous q-slice, spread over
                # t=1..nkt-1 (fillers at t=0 would wait on the fresh drain)
                pending = list(range(NDP)) if qs > 0 else []
                reserve = 2 if pending else 0
                navail = len(pending) - reserve
                for t in range(nkt):
                    n_fill = 0
                    if pending and t > 0:
                        n_fill = (t * navail) // (nkt - 1) \
                            - ((t - 1) * navail) // (nkt - 1)
                    delta = t * 128 - qs * QSL
                    dd = max(delta, 0)
                    ktile = kT[:, t * 128:(t + 1) * 128]
                    qslc = slice(qs * QSL + dd, (qs + 1) * QSL)
                    if dd >= 256:
                        # narrow diagonal tile: pack all 4 heads' trimmed
                        # scores in ONE slot (halves ring pressure, one exp)
                        w = 512 - dd
                        ps = trS.tile([128, 1024], F32, tag="trS", name="psD")
                        for g in range(HQ):
                            nc.tensor.matmul(ps[:, g * w:(g + 1) * w],
                                             ktile, qT[g][:, qslc],
                                             start=True, stop=True)
                        for g in range(HQ):
                            blk = slice(g * w, g * w + 128)
                            nc.vector.tensor_add(ps[:, blk], ps[:, blk],
                                                 mneg_sb)
                        ex = ep.tile([128, 1024], F16, tag="ex")
                        nc.scalar.activation(ex[:, 0:4 * w], ps[:, 0:4 * w],
                                             Exp, bias=cb_sb, scale=scale)
                        this_pv = (po, "packed", (ex, w), t, dd, nkt)
                    else:
                        exs = []
                        for p in range(2):
                            ps = trS.tile([128, 1024], F32, tag="trS",
                                          name="psS")
                            for i in range(2):
                                g = 2 * p + i
                                nc.tensor.matmul(
                                    ps[:, i * 512 + dd:(i + 1) * 512],
                                    ktile, qT[g][:, qslc],
                                    start=True, stop=True)
                            if delta >= 0:
                                for i in range(2):
                                    blk = slice(i * 512 + dd,
                                                i * 512 + dd + 128)
                                    nc.vector.tensor_add(ps[:, blk],
                                                         ps[:, blk], mneg_sb)
                            ex = ep.tile([128, 1024], F16, tag="ex")
                            # dd=128: full-pair exp is as cheap as two trimmed
                            # ops; the stale strips are never read (pv and the
                            # denominator adds are trimmed to [dd:512])
                            nc.scalar.activation(ex, ps, Exp,
                                                 bias=cb_sb, scale=scale)
                            exs.append(ex)
                        this_pv = (po, "pair", exs, t, dd, nkt)
                    # previous tile's pv: independent PE work that covers the
                    # exp latency and the score-slot WAR on the in-order PE
                    if pend_pv is not None:
                        emit_pv(pend_pv)
                    for _ in range(n_fill):
                        if len(pending) > reserve:
                            emit_pf_pair(qs - 1, pending.pop(0))
                    pend_pv = this_pv
                    for p in range(2):
                        if t == 0:
                            nc.vector.tensor_copy(acc[p], exs[p])
                        elif dd == 0:
                            nc.vector.tensor_add(acc[p], acc[p], exs[p])
                        elif dd >= 256:
                            for i in range(2):
                                h = slice(i * 512 + dd, (i + 1) * 512)
                                g = 2 * p + i
                                nc.vector.tensor_add(acc[p][:, h],
                                                     acc[p][:, h],
                                                     ex[:, g * w:(g + 1) * w])
                        else:
                            for i in range(2):
                                h = slice(i * 512 + dd, (i + 1) * 512)
                                nc.vector.tensor_add(acc[p][:, h],
                                                     acc[p][:, h],
                                                     exs[p][:, h])

                # denominators (need only the DVE acc chain, not the pvs)
                dns = []
                pds = []
                for p in range(2):
                    pd = trS.tile([128, 1024], F32, tag="trS", name="pd")
                    for i in range(2):
                        h = slice(i * 512, (i + 1) * 512)
                        nc.tensor.matmul(pd[:, h], ones_sb, acc[p][:, h],
                                         start=True, stop=True)
                    pds.append(pd)
                # last tile's pv before the reciprocal chain needs it
                emit_pv(pend_pv)
                pend_pv = None
                pdcs = []
                for p in range(2):
                    # ACT copy releases the PSUM slot and runs parallel to
                    # the DVE reciprocal/normalize chain
                    pdc = dp.tile([128, 1024], F32, tag="pdc")
                    nc.scalar.copy(pdc, pds[p])
                    pdcs.append(pdc)
                for p in range(2):
                    dn = dp.tile([128, 1024], F32, tag="dn")
                    nc.vector.reciprocal_approx_fast(out=dn, in_=pdcs[p])
                    for i in range(2):
                        g = 2 * p + i
                        h = slice(i * 512, (i + 1) * 512)
                        nc.vector.tensor_mul(o_attn[g][:, qsl],
                                             po[p][:, h], dn[:, h])
                # reserved PE filler covers the reciprocal+drain latency
                while pending:
                    emit_pf_pair(qs - 1, pending.pop(0))

            for dpair in range(NDP):
                emit_pf_pair(NQS - 1, dpair)

    nc.compile()
    return nc


def make_tables(cfg: Cfg, position_ids: np.ndarray):
    """cos/sin [128, S] fp16: row d holds cos/sin(pos * invfreq[d % 64])."""
    half = cfg.DH // 2
    inv = 1.0 / (cfg.theta ** (np.arange(half, dtype=np.float64) * 2.0 / cfg.DH))
    pos = np.asarray(position_ids).reshape(-1).astype(np.float64)
    ang = inv[:, None] * pos[None, :]
    cosT = np.concatenate([np.cos(ang), np.cos(ang)], 0).astype(np.float16)
    sinT = np.concatenate([np.sin(ang), np.sin(ang)], 0).astype(np.float16)
    return cosT, sinT


def make_rotm(cfg: Cfg):
    """[128, 256] fp16: cols 0-127 rotate-half matrix (out = M^T @ x ->
    out[:64] = -x[64:], out[64:] = x[:64]); cols 128-255 all ones."""
    half = cfg.DH // 2
    m = np.zeros((128, 256), np.float16)
    for i in range(half):
        m[i + half, i] = -1.0
        m[i, i + half] = 1.0
    m[:, 128:] = 1.0
    return m


def make_mneg():
    """[128, 128] f32 strict lower-triangular NEG: mask[p, j] = NEG if j < p."""
    j = np.arange(128)[None, :]
    p = np.arange(128)[:, None]
    return np.where(j < p, NEG, 0.0).astype(np.float32)


_cache = threading.Lock()
_nc_full = None


def _get_nc():
    global _nc_full
    with _cache:
        if _nc_full is None:
            _nc_full = build_nc(FULL)
    return _nc_full


def _prep_shared(cfg: Cfg, position_ids, hidden_states):
    S, D, DT = cfg.S, cfg.D, cfg.D // 128
    h16 = np.asarray(hidden_states, np.float32).reshape(S, D).T.astype(np.float16)
    # [D, S] -> [2*NSL half-slices, 128 partitions, DT*256]
    ht = np.ascontiguousarray(
        h16.reshape(DT, 128, 2 * (S // 512), 256).transpose(2, 1, 0, 3)
        .reshape(2 * (S // 512), 128, DT * 256))
    cosT, sinT = make_tables(cfg, position_ids)
    return ht, cosT, sinT


def core_inputs(cfg: Cfg, c: int, shared, Wq, Wk, Wv, Wo):
    S, D, HQ, DH = cfg.S, cfg.D, cfg.HQ, cfg.DH
    DT = D // 128
    ht, cosT, sinT = shared
    qc = slice(c * HQ * DH, (c + 1) * HQ * DH)
    kc = slice(c * DH, (c + 1) * DH)

    def wtile(w, cols, n):
        w16 = np.asarray(w, np.float32)[:, cols].astype(np.float16)
        return np.ascontiguousarray(w16.reshape(DT, 128, n).transpose(1, 0, 2))

    wo16 = np.asarray(Wo, np.float32)[qc, :].astype(np.float16)
    wo_t = np.ascontiguousarray(wo16.reshape(HQ, 128, D).transpose(1, 0, 2))
    return {
        "ht": ht,
        "wq": wtile(Wq, qc, HQ * DH),
        "wk": wtile(Wk, kc, DH),
        "wv": wtile(Wv, kc, DH),
        "wo": wo_t,
        "cosh": cosT,
        "sinh": sinT,
        "mneg": make_mneg(),
        "rotm": make_rotm(cfg),
    }


def kernel(position_ids, hidden_states, Wq, Wk, Wv, Wo, _trace=False):
    from concourse.bass_utils import run_bass_kernel_spmd

    cfg = FULL
    nc = _get_nc()
    shared = _prep_shared(cfg, position_ids, hidden_states)
    in_maps = [core_inputs(cfg, c, shared, Wq, Wk, Wv, Wo)
               for c in range(cfg.cores)]
    res = run_bass_kernel_spmd(nc, in_maps, core_ids=list(range(cfg.cores)),
                               trace=_trace)
    out = np.zeros((cfg.S, cfg.D), np.float64)
    for c in range(cfg.cores):
        out += res.results[c]["outT"].T.astype(np.float64)
    ret = out.astype(np.float32).reshape(1, cfg.S, cfg.D)
    if _trace:
        return ret, res
    return ret


# revision 29
# speedup vs baseline: 1.0077x; 1.0019x over previous
"""Trainium2 Bass kernel for Llama-style GQA attention (B=1, S=2048, D=4096,
32 q heads / 8 kv heads, head_dim 128, neox RoPE, causal).

Sharding: tensor-parallel over kv heads across 8 NeuronCores. Core c owns
kv head c and q heads [4c, 4c+4): Wq cols [512c, 512c+512), Wk/Wv cols
[128c, 128c+128), Wo rows [512c, 512c+512). Each core computes a full
[D, S] partial of the output (o_proj row-parallel); host sums the 8 partials.

All matmul data is fp16 (PSUM accumulation f32); inputs are cast on the host.
Per-core kernel:
  A) Per s-slice of 512 (two DMA half-slices of 256): project q0..q3/k
     transposed ([dh, s], weights stationary, hiddenT moving) and v in
     natural [s, dh] layout (hiddenT-block stationary, Wv moving; four
     128-row groups packed per PSUM bank). RoPE is fused per-slice: ACT
     drains PSUM->fp16, a rotate-half PE matmul, then DVE fp16 combines
     x*cos + rot(x)*sin. PE never idles between slices.
  B) Attention per q-slice of 512, interleaved with the previous q-slice's
     o_proj matmuls as PE filler: scoresT pairs two heads in one
     [128,1024] PSUM tile -> one Exp per pair (bias=-4 keeps exp in fp16
     range; softmax is shift-invariant), diagonal tiles width-trimmed with
     a [128,128] triangular mask add, ex in fp16. pv accumulates O^T in
     paired PSUM. Softmax denominators: DVE fp16 accumulate of ex tiles,
     then a ones-stationary matmul replicates the partition sum; normalize
     happens in the PSUM->SBUF drain mul.
  C) o_proj: out^T[D,S] partial, Wo stationary, O^T moving, PSUM pairs
     (two 128-row D blocks) -> fp16 SBUF -> DRAM.
"""

import threading
from dataclasses import dataclass

import numpy as np


@dataclass(frozen=True)
class Cfg:
    S: int = 2048
    D: int = 4096
    HQ: int = 4        # q heads per core
    DH: int = 128
    QSL: int = 512     # q-slice width for attention
    theta: float = 10000.0
    cores: int = 8


FULL = Cfg()
NEG = -1.0e9
CBIAS = -6.0  # exp(scale*score + CBIAS): keeps exp sums in fp16 range


def build_nc(cfg: Cfg):
    import concourse.bass as bass  # noqa: F401
    import concourse.mybir as mybir
    import concourse.tile as tile
    from concourse import bacc
    from concourse.masks import make_identity  # noqa: F401

    F16 = mybir.dt.float16
    F32 = mybir.dt.float32
    Exp = mybir.ActivationFunctionType.Exp

    S, D, HQ, DH, QSL = cfg.S, cfg.D, cfg.HQ, cfg.DH, cfg.QSL
    DT = D // 128            # contraction d-tiles
    NSL = S // 512           # 512-wide s-slices
    NKT = S // 128           # k-position tiles
    NQS = S // QSL           # q slices
    NDP = DT // 2            # o_proj D-block pairs
    scale = float(DH) ** -0.5

    nc = bacc.Bacc("TRN2", target_bir_lowering=False, debug=False,
                   num_devices=cfg.cores)

    ht = nc.dram_tensor("ht", [2 * NSL, 128, DT * 256], F16,
                        kind="ExternalInput").ap()
    wq = nc.dram_tensor("wq", [128, DT, HQ * DH], F16,
                        kind="ExternalInput").ap()
    wk = nc.dram_tensor("wk", [128, DT, DH], F16, kind="ExternalInput").ap()
    wv = nc.dram_tensor("wv", [128, DT, DH], F16, kind="ExternalInput").ap()
    wo = nc.dram_tensor("wo", [128, HQ, D], F16, kind="ExternalInput").ap()
    cosh = nc.dram_tensor("cosh", [128, S], F16, kind="ExternalInput").ap()
    sinh = nc.dram_tensor("sinh", [128, S], F16, kind="ExternalInput").ap()
    mneg = nc.dram_tensor("mneg", [128, 128], F32, kind="ExternalInput").ap()
    rotm = nc.dram_tensor("rotm", [128, 256], F16, kind="ExternalInput").ap()
    outT = nc.dram_tensor("outT", [D, S], F16, kind="ExternalOutput").ap()

    with tile.TileContext(nc) as tc, \
            tc.tile_pool(name="persist", bufs=1) as pp:
        qT = [pp.tile([128, S], F16, tag=f"qT{g}", name=f"qT{g}")
              for g in range(HQ)]
        kT = pp.tile([128, S], F16, tag="kT")
        v_all = pp.tile([128, NKT, DH], F16, tag="vall")
        o_attn = [pp.tile([128, S], F16, tag=f"oT{g}", name=f"oT{g}")
                  for g in range(HQ)]
        cos_sb = pp.tile([128, S], F16, tag="cos")
        sin_sb = pp.tile([128, S], F16, tag="sin")
        mneg_sb = pp.tile([128, 128], F32, tag="mneg")
        rotm_sb = pp.tile([128, 256], F16, tag="rotm")
        wo_sb = pp.tile([128, HQ, D], F16, tag="wo")
        rot_sb = rotm_sb[:, 0:128]
        ones_sb = rotm_sb[:, 128:256]

        cb_sb = pp.tile([128, 1], F32, tag="cbias")
        nc.gpsimd.memset(cb_sb, CBIAS)
        nc.sync.dma_start(out=mneg_sb, in_=mneg)
        nc.sync.dma_start(out=rotm_sb, in_=rotm)

        # ---------------- phase A: projections + RoPE ----------------
        with (
            tc.tile_pool(name="wpool", bufs=1) as wp,
            tc.tile_pool(name="htp", bufs=3) as hp,
            tc.tile_pool(name="psA", bufs=6, space="PSUM") as psA,
            tc.tile_pool(name="trA", bufs=2, space="PSUM") as trA,
            tc.tile_pool(name="rawp", bufs=5) as rp,
            tc.tile_pool(name="prhp", bufs=2) as php,
            tc.tile_pool(name="rtmp", bufs=4) as rtp,
        ):
            wk_sb = wp.tile([128, DT, DH], F16, tag="wk")
            wv_sb = wp.tile([128, DT, DH], F16, tag="wv")
            wq_sb = wp.tile([128, DT, HQ * DH], F16, tag="wq")

            # DMA order: wk/ht0 in fine chunks so the k-pass starts almost
            # immediately, then wq in d-chunks, wv, tables, wo prefetch.
            ht_tiles = {}

            def ht_dma(hs, chunks=1):
                t = hp.tile([128, DT * 256], F16, tag="ht", name=f"ht{hs}")
                cw = DT * 256 // chunks
                for ci in range(chunks):
                    nc.sync.dma_start(out=t[:, ci * cw:(ci + 1) * cw],
                                      in_=ht[hs][:, ci * cw:(ci + 1) * cw])
                ht_tiles[hs] = t

            nc.sync.dma_start(out=wk_sb[:, 0:8, :], in_=wk[:, 0:8, :])
            ht_dma(0, chunks=8)
            nc.sync.dma_start(out=wk_sb[:, 8:32, :], in_=wk[:, 8:32, :])
            nc.sync.dma_start(out=wq_sb[:, 0:8, :], in_=wq[:, 0:8, :])
            nc.sync.dma_start(out=wq_sb[:, 8:16, :], in_=wq[:, 8:16, :])
            nc.sync.dma_start(out=wv_sb, in_=wv)
            ht_dma(1)
            nc.sync.dma_start(out=wq_sb[:, 16:24, :], in_=wq[:, 16:24, :])
            nc.sync.dma_start(out=wq_sb[:, 24:32, :], in_=wq[:, 24:32, :])
            nc.sync.dma_start(out=cos_sb, in_=cosh)
            nc.sync.dma_start(out=sin_sb, in_=sinh)
            ht_dma(2)
            nc.sync.dma_start(out=wo_sb, in_=wo)

            for sl in range(NSL):
                ssl = slice(sl * 512, (sl + 1) * 512)
                psQ = [psA.tile([128, 512], F32, tag="psA", name=f"psQ{g}")
                       for g in range(HQ)]
                psK = psA.tile([128, 512], F32, tag="psA", name="psK")
                psV = psA.tile([128, 512], F32, tag="psA", name="psV")
                def kpass(ht_t, hcols):
                    for d in range(DT):
                        nc.tensor.matmul(
                            psK[:, hcols], wk_sb[:, d, :],
                            ht_t[:, d * 256:(d + 1) * 256],
                            start=(d == 0), stop=(d == DT - 1))

                def qpass(ht_t, hcols):
                    for d in range(DT):
                        mv = ht_t[:, d * 256:(d + 1) * 256]
                        for g in range(HQ):
                            nc.tensor.matmul(
                                psQ[g][:, hcols],
                                wq_sb[:, d, g * DH:(g + 1) * DH], mv,
                                start=(d == 0), stop=(d == DT - 1))

                def vpass(ht_t, h):
                    # natural layout, hiddenT block stationary
                    for sb in range(2):
                        vc = slice((2 * h + sb) * 128, (2 * h + sb + 1) * 128)
                        for d in range(DT):
                            nc.tensor.matmul(
                                psV[:, vc],
                                ht_t[:, d * 256 + sb * 128:
                                     d * 256 + (sb + 1) * 128],
                                wv_sb[:, d, :],
                                start=(d == 0), stop=(d == DT - 1))

                def rope(raw, dstT):
                    pr = trA.tile([128, 512], F32, tag="pr", name="pr")
                    nc.tensor.matmul(pr, rot_sb, raw, start=True, stop=True)
                    prh = php.tile([128, 512], F16, tag="prh")
                    nc.scalar.copy(prh, pr)
                    rs = rtp.tile([128, 512], F16, tag="rs")
                    nc.vector.tensor_mul(rs, prh, sin_sb[:, ssl])
                    cc = rtp.tile([128, 512], F16, tag="cc")
                    nc.vector.tensor_mul(cc, raw, cos_sb[:, ssl])
                    nc.vector.tensor_add(dstT[:, ssl], cc, rs)

                def ht_get(h):
                    hs = 2 * sl + h
                    if hs + 2 < 2 * NSL and (hs + 2) not in ht_tiles:
                        ht_dma(hs + 2)
                    return ht_tiles.pop(hs)

                # h0: k first (its weights arrive first); h1: q first with
                # eager drains so the PSUM accumulators release while the
                # k/v passes still run on the PE (faster A->C transition)
                ht0 = ht_get(0)
                kpass(ht0, slice(0, 256))
                qpass(ht0, slice(0, 256))
                vpass(ht0, 0)
                ht1 = ht_get(1)
                qpass(ht1, slice(256, 512))
                qraws = []
                for g in range(HQ):
                    qr = rp.tile([128, 512], F16, tag="raw", name=f"qraw{g}")
                    nc.scalar.copy(qr, psQ[g])
                    qraws.append(qr)
                kpass(ht1, slice(256, 512))
                for g in range(HQ):
                    rope(qraws[g], qT[g])
                kr = rp.tile([128, 512], F16, tag="raw", name="kraw")
                nc.scalar.copy(kr, psK)
                vpass(ht1, 1)
                rope(kr, kT)
                nc.scalar.copy(v_all[:, 4 * sl:4 * sl + 4, :], psV)

        # ---------------- phase B+C: attention + o_proj ----------------
        outT_r = outT.rearrange("(x p) s -> p x s", p=128)
        with (
            tc.tile_pool(name="psO", bufs=2, space="PSUM") as psO,
            tc.tile_pool(name="trS", bufs=2, space="PSUM") as trS,
            tc.tile_pool(name="expp", bufs=6) as ep,
            tc.tile_pool(name="accp", bufs=4) as ap_,
            tc.tile_pool(name="dnp", bufs=4) as dp,
            tc.tile_pool(name="ocp", bufs=4) as ocp,
        ):
            def emit_pf_pair(qs, dpair):
                qsl = slice(qs * QSL, (qs + 1) * QSL)
                pf = trS.tile([128, 1024], F32, tag="trS", name="pf")
                for g in range(HQ):
                    for i in range(2):
                        nc.tensor.matmul(
                            pf[:, i * 512:(i + 1) * 512],
                            wo_sb[:, g, (2 * dpair + i) * 128:
                                  (2 * dpair + i + 1) * 128],
                            o_attn[g][:, qsl],
                            start=(g == 0), stop=(g == HQ - 1))
                oc = ocp.tile([128, 1024], F16, tag="oc")
                # parallel half-casts (ACT + DVE) release the PSUM slot fast
                nc.scalar.copy(oc[:, 0:512], pf[:, 0:512])
                nc.vector.tensor_copy(oc[:, 512:1024], pf[:, 512:1024])
                nc.sync.dma_start(
                    out=outT_r[:, 2 * dpair:2 * dpair + 2, qsl], in_=oc)

            pend_pv = None  # pv of the previous tile, emitted one tile late

            def emit_pv(pv):
                po, kind, exd, t, dd, nkt = pv
                for p in range(2):
                    for i in range(2):
                        h = slice(i * 512 + dd, (i + 1) * 512)
                        if kind == "pair":
                            mv = exd[p][:, h]
                        else:  # packed: 4 heads side by side, width w each
                            ex, w = exd
                            g = 2 * p + i
                            mv = ex[:, g * w:(g + 1) * w]
                        nc.tensor.matmul(po[p][:, h], v_all[:, t, :], mv,
                                         start=(t == 0),
                                         stop=(t == nkt - 1))

            for qs in range(NQS):
                qsl = slice(qs * QSL, (qs + 1) * QSL)
                nkt = 4 * (qs + 1)
                po = [psO.tile([128, 1024], F32, tag="po", name=f"po{p}")
                      for p in range(2)]
                acc = [ap_.tile([128, 1024], F16, tag="acc", name=f"acc{p}")
                       for p in range(2)]
                # o_proj filler work from the previous q-slice, spread over
                # t=1..nkt-1 (fillers at t=0 would wait on the fresh drain)
                pending = list(range(NDP)) if qs > 0 else []
                reserve = 3 if pending else 0
                navail = len(pending) - reserve
                for t in range(nkt):
                    n_fill = 0
                    if pending and t > 0:
                        n_fill = (t * navail) // (nkt - 1) \
                            - ((t - 1) * navail) // (nkt - 1)
                    delta = t * 128 - qs * QSL
                    dd = max(delta, 0)
                    ktile = kT[:, t * 128:(t + 1) * 128]
                    qslc = slice(qs * QSL + dd, (qs + 1) * QSL)
                    if dd >= 256:
                        # narrow diagonal tile: pack all 4 heads' trimmed
                        # scores in ONE slot (halves ring pressure, one exp)
                        w = 512 - dd
                        ps = trS.tile([128, 1024], F32, tag="trS", name="psD")
                        for g in range(HQ):
                            nc.tensor.matmul(ps[:, g * w:(g + 1) * w],
                                             ktile, qT[g][:, qslc],
                                             start=True, stop=True)
                        for g in range(HQ):
                            blk = slice(g * w, g * w + 128)
                            nc.vector.tensor_add(ps[:, blk], ps[:, blk],
                                                 mneg_sb)
                        ex = ep.tile([128, 1024], F16, tag="ex")
                        nc.scalar.activation(ex[:, 0:4 * w], ps[:, 0:4 * w],
                                             Exp, bias=cb_sb, scale=scale)
                        this_pv = (po, "packed", (ex, w), t, dd, nkt)
                    else:
                        exs = []
                        for p in range(2):
                            ps = trS.tile([128, 1024], F32, tag="trS",
                                          name="psS")
                            for i in range(2):
                                g = 2 * p + i
                                nc.tensor.matmul(
                                    ps[:, i * 512 + dd:(i + 1) * 512],
                                    ktile, qT[g][:, qslc],
                                    start=True, stop=True)
                            if delta >= 0:
                                for i in range(2):
                                    blk = slice(i * 512 + dd,
                                                i * 512 + dd + 128)
                                    nc.vector.tensor_add(ps[:, blk],
                                                         ps[:, blk], mneg_sb)
                            ex = ep.tile([128, 1024], F16, tag="ex")
                            # dd=128: full-pair exp is as cheap as two trimmed
                            # ops; the stale strips are never read (pv and the
                            # denominator adds are trimmed to [dd:512])
                            nc.scalar.activation(ex, ps, Exp,
                                                 bias=cb_sb, scale=scale)
                            exs.append(ex)
                        this_pv = (po, "pair", exs, t, dd, nkt)
                    # previous tile's pv: independent PE work that covers the
                    # exp latency and the score-slot WAR on the in-order PE
                    if pend_pv is not None:
                        emit_pv(pend_pv)
                    for _ in range(n_fill):
                        if len(pending) > reserve:
                            emit_pf_pair(qs - 1, pending.pop(0))
                    pend_pv = this_pv
                    # ping-pong accumulate (out != in0 keeps DVE fast modes)
                    for p in range(2):
                        if t == 0:
                            nc.vector.tensor_copy(acc[p], exs[p])
                        elif dd == 0:
                            na = ap_.tile([128, 1024], F16, tag="acc",
                                          name=f"acc{p}")
                            nc.vector.tensor_add(na, acc[p], exs[p])
                            acc[p] = na
                        elif dd >= 256:
                            for i in range(2):
                                h = slice(i * 512 + dd, (i + 1) * 512)
                                g = 2 * p + i
                                nc.vector.tensor_add(acc[p][:, h],
                                                     acc[p][:, h],
                                                     ex[:, g * w:(g + 1) * w])
                        else:
                            for i in range(2):
                                h = slice(i * 512 + dd, (i + 1) * 512)
                                nc.vector.tensor_add(acc[p][:, h],
                                                     acc[p][:, h],
                                                     exs[p][:, h])

                # denominators (need only the DVE acc chain, not the pvs)
                dns = []
                pds = []
                for p in range(2):
                    pd = trS.tile([128, 1024], F32, tag="trS", name="pd")
                    for i in range(2):
                        h = slice(i * 512, (i + 1) * 512)
                        nc.tensor.matmul(pd[:, h], ones_sb, acc[p][:, h],
                                         start=True, stop=True)
                    pds.append(pd)
                # last tile's pv before the reciprocal chain needs it
                emit_pv(pend_pv)
                pend_pv = None
                pdcs = []
                for p in range(2):
                    # ACT copy releases the PSUM slot and runs parallel to
                    # the DVE reciprocal/normalize chain
                    pdc = dp.tile([128, 1024], F32, tag="pdc")
                    nc.scalar.copy(pdc, pds[p])
                    pdcs.append(pdc)
                for p in range(2):
                    dn = dp.tile([128, 1024], F32, tag="dn")
                    nc.vector.reciprocal_approx_fast(out=dn, in_=pdcs[p])
                    for i in range(2):
                        g = 2 * p + i
                        h = slice(i * 512, (i + 1) * 512)
                        nc.vector.tensor_mul(o_attn[g][:, qsl],
                                             po[p][:, h], dn[:, h])
                # reserved PE filler covers the reciprocal+drain latency
                while pending:
                    emit_pf_pair(qs - 1, pending.pop(0))

            for dpair in range(NDP):
                emit_pf_pair(NQS - 1, dpair)

    nc.compile()
    return nc


def make_tables(cfg: Cfg, position_ids: np.ndarray):
    """cos/sin [128, S] fp16: row d holds cos/sin(pos * invfreq[d % 64])."""
    half = cfg.DH // 2
    inv = 1.0 / (cfg.theta ** (np.arange(half, dtype=np.float64) * 2.0 / cfg.DH))
    pos = np.asarray(position_ids).reshape(-1).astype(np.float64)
    ang = inv[:, None] * pos[None, :]
    cosT = np.concatenate([np.cos(ang), np.cos(ang)], 0).astype(np.float16)
    sinT = np.concatenate([np.sin(ang), np.sin(ang)], 0).astype(np.float16)
    return cosT, sinT


def make_rotm(cfg: Cfg):
    """[128, 256] fp16: cols 0-127 rotate-half matrix (out = M^T @ x ->
    out[:64] = -x[64:], out[64:] = x[:64]); cols 128-255 all ones."""
    half = cfg.DH // 2
    m = np.zeros((128, 256), np.float16)
    for i in range(half):
        m[i + half, i] = -1.0
        m[i, i + half] = 1.0
    m[:, 128:] = 1.0
    return m


def make_mneg():
    """[128, 128] f32 strict lower-triangular NEG: mask[p, j] = NEG if j < p."""
    j = np.arange(128)[None, :]
    p = np.arange(128)[:, None]
    return np.where(j < p, NEG, 0.0).astype(np.float32)


_cache = threading.Lock()
_nc_full = None


def _get_nc():
    global _nc_full
    with _cache:
        if _nc_full is None:
            _nc_full = build_nc(FULL)
    return _nc_full


def _prep_shared(cfg: Cfg, position_ids, hidden_states):
    S, D, DT = cfg.S, cfg.D, cfg.D // 128
    h16 = np.asarray(hidden_states, np.float32).reshape(S, D).T.astype(np.float16)
    # [D, S] -> [2*NSL half-slices, 128 partitions, DT*256]
    ht = np.ascontiguousarray(
        h16.reshape(DT, 128, 2 * (S // 512), 256).transpose(2, 1, 0, 3)
        .reshape(2 * (S // 512), 128, DT * 256))
    cosT, sinT = make_tables(cfg, position_ids)
    return ht, cosT, sinT


def core_inputs(cfg: Cfg, c: int, shared, Wq, Wk, Wv, Wo):
    S, D, HQ, DH = cfg.S, cfg.D, cfg.HQ, cfg.DH
    DT = D // 128
    ht, cosT, sinT = shared
    qc = slice(c * HQ * DH, (c + 1) * HQ * DH)
    kc = slice(c * DH, (c + 1) * DH)

    def wtile(w, cols, n):
        w16 = np.asarray(w, np.float32)[:, cols].astype(np.float16)
        return np.ascontiguousarray(w16.reshape(DT, 128, n).transpose(1, 0, 2))

    wo16 = np.asarray(Wo, np.float32)[qc, :].astype(np.float16)
    wo_t = np.ascontiguousarray(wo16.reshape(HQ, 128, D).transpose(1, 0, 2))
    return {
        "ht": ht,
        "wq": wtile(Wq, qc, HQ * DH),
        "wk": wtile(Wk, kc, DH),
        "wv": wtile(Wv, kc, DH),
        "wo": wo_t,
        "cosh": cosT,
        "sinh": sinT,
        "mneg": make_mneg(),
        "rotm": make_rotm(cfg),
    }


def kernel(position_ids, hidden_states, Wq, Wk, Wv, Wo, _trace=False):
    from concourse.bass_utils import run_bass_kernel_spmd

    cfg = FULL
    nc = _get_nc()
    shared = _prep_shared(cfg, position_ids, hidden_states)
    in_maps = [core_inputs(cfg, c, shared, Wq, Wk, Wv, Wo)
               for c in range(cfg.cores)]
    res = run_bass_kernel_spmd(nc, in_maps, core_ids=list(range(cfg.cores)),
                               trace=_trace)
    out = np.zeros((cfg.S, cfg.D), np.float64)
    for c in range(cfg.cores):
        out += res.results[c]["outT"].T.astype(np.float64)
    ret = out.astype(np.float32).reshape(1, cfg.S, cfg.D)
    if _trace:
        return ret, res
    return ret


# revision 30
# speedup vs baseline: 1.0145x; 1.0068x over previous
"""Trainium2 Bass kernel for Llama-style GQA attention (B=1, S=2048, D=4096,
32 q heads / 8 kv heads, head_dim 128, neox RoPE, causal).

Sharding: tensor-parallel over kv heads across 8 NeuronCores. Core c owns
kv head c and q heads [4c, 4c+4): Wq cols [512c, 512c+512), Wk/Wv cols
[128c, 128c+128), Wo rows [512c, 512c+512). Each core computes a full
[D, S] partial of the output (o_proj row-parallel); host sums the 8 partials.

All matmul data is fp16 (PSUM accumulation f32); inputs are cast on the host.
Per-core kernel:
  A) Per s-slice of 512 (two DMA half-slices of 256): project q0..q3/k
     transposed ([dh, s], weights stationary, hiddenT moving) and v in
     natural [s, dh] layout (hiddenT-block stationary, Wv moving; four
     128-row groups packed per PSUM bank). RoPE is fused per-slice: ACT
     drains PSUM->fp16, a rotate-half PE matmul, then DVE fp16 combines
     x*cos + rot(x)*sin. PE never idles between slices.
  B) Attention per q-slice of 512, interleaved with the previous q-slice's
     o_proj matmuls as PE filler: scoresT pairs two heads in one
     [128,1024] PSUM tile -> one Exp per pair (bias=-4 keeps exp in fp16
     range; softmax is shift-invariant), diagonal tiles width-trimmed with
     a [128,128] triangular mask add, ex in fp16. pv accumulates O^T in
     paired PSUM. Softmax denominators: DVE fp16 accumulate of ex tiles,
     then a ones-stationary matmul replicates the partition sum; normalize
     happens in the PSUM->SBUF drain mul.
  C) o_proj: out^T[D,S] partial, Wo stationary, O^T moving, PSUM pairs
     (two 128-row D blocks) -> fp16 SBUF -> DRAM.
"""

import threading
from dataclasses import dataclass

import numpy as np


@dataclass(frozen=True)
class Cfg:
    S: int = 2048
    D: int = 4096
    HQ: int = 4        # q heads per core
    DH: int = 128
    QSL: int = 512     # q-slice width for attention
    theta: float = 10000.0
    cores: int = 8


FULL = Cfg()
NEG = -1.0e9
CBIAS = -6.0  # exp(scale*score + CBIAS): keeps exp sums in fp16 range


def build_nc(cfg: Cfg):
    import concourse.bass as bass  # noqa: F401
    import concourse.mybir as mybir
    import concourse.tile as tile
    from concourse import bacc
    from concourse.masks import make_identity  # noqa: F401

    F16 = mybir.dt.float16
    F32 = mybir.dt.float32
    Exp = mybir.ActivationFunctionType.Exp

    S, D, HQ, DH, QSL = cfg.S, cfg.D, cfg.HQ, cfg.DH, cfg.QSL
    DT = D // 128            # contraction d-tiles
    NSL = S // 512           # 512-wide s-slices
    NKT = S // 128           # k-position tiles
    NQS = S // QSL           # q slices
    NDP = DT // 2            # o_proj D-block pairs
    scale = float(DH) ** -0.5

    nc = bacc.Bacc("TRN2", target_bir_lowering=False, debug=False,
                   num_devices=cfg.cores)

    ht = nc.dram_tensor("ht", [2 * NSL, 128, DT * 256], F16,
                        kind="ExternalInput").ap()
    wq = nc.dram_tensor("wq", [128, DT, HQ * DH], F16,
                        kind="ExternalInput").ap()
    wk = nc.dram_tensor("wk", [128, DT, DH], F16, kind="ExternalInput").ap()
    wv = nc.dram_tensor("wv", [128, DT, DH], F16, kind="ExternalInput").ap()
    wo = nc.dram_tensor("wo", [128, HQ, D], F16, kind="ExternalInput").ap()
    cosh = nc.dram_tensor("cosh", [128, S], F16, kind="ExternalInput").ap()
    sinh = nc.dram_tensor("sinh", [128, S], F16, kind="ExternalInput").ap()
    mneg = nc.dram_tensor("mneg", [128, 128], F32, kind="ExternalInput").ap()
    rotm = nc.dram_tensor("rotm", [128, 256], F16, kind="ExternalInput").ap()
    outT = nc.dram_tensor("outT", [D, S], F16, kind="ExternalOutput").ap()

    with tile.TileContext(nc) as tc, \
            tc.tile_pool(name="persist", bufs=1) as pp:
        qT = [pp.tile([128, S], F16, tag=f"qT{g}", name=f"qT{g}")
              for g in range(HQ)]
        kT = pp.tile([128, S], F16, tag="kT")
        v_all = pp.tile([128, NKT, DH], F16, tag="vall")
        o_attn = [pp.tile([128, S], F16, tag=f"oT{g}", name=f"oT{g}")
                  for g in range(HQ)]
        cos_sb = pp.tile([128, S], F16, tag="cos")
        sin_sb = pp.tile([128, S], F16, tag="sin")
        mneg_sb = pp.tile([128, 128], F32, tag="mneg")
        rotm_sb = pp.tile([128, 256], F16, tag="rotm")
        wo_sb = pp.tile([128, HQ, D], F16, tag="wo")
        rot_sb = rotm_sb[:, 0:128]
        ones_sb = rotm_sb[:, 128:256]

        cb_sb = pp.tile([128, 1], F32, tag="cbias")
        nc.gpsimd.memset(cb_sb, CBIAS)
        nc.sync.dma_start(out=mneg_sb, in_=mneg)
        nc.sync.dma_start(out=rotm_sb, in_=rotm)

        # ---------------- phase A: projections + RoPE ----------------
        with (
            tc.tile_pool(name="wpool", bufs=1) as wp,
            tc.tile_pool(name="htp", bufs=3) as hp,
            tc.tile_pool(name="psA", bufs=6, space="PSUM") as psA,
            tc.tile_pool(name="trA", bufs=2, space="PSUM") as trA,
            tc.tile_pool(name="rawp", bufs=5) as rp,
            tc.tile_pool(name="prhp", bufs=2) as php,
            tc.tile_pool(name="rtmp", bufs=4) as rtp,
        ):
            wk_sb = wp.tile([128, DT, DH], F16, tag="wk")
            wv_sb = wp.tile([128, DT, DH], F16, tag="wv")
            wq_sb = wp.tile([128, DT, HQ * DH], F16, tag="wq")

            # DMA order: wk/ht0 in fine chunks so the k-pass starts almost
            # immediately, then wq in d-chunks, wv, tables, wo prefetch.
            ht_tiles = {}

            def ht_dma(hs, chunks=1):
                t = hp.tile([128, DT * 256], F16, tag="ht", name=f"ht{hs}")
                cw = DT * 256 // chunks
                for ci in range(chunks):
                    nc.sync.dma_start(out=t[:, ci * cw:(ci + 1) * cw],
                                      in_=ht[hs][:, ci * cw:(ci + 1) * cw])
                ht_tiles[hs] = t

            # interleave triggers in consumption order: the sync engine
            # serializes dma_starts at ~0.6us each, so few coarse chunks
            # beat many fine ones
            ht0 = hp.tile([128, DT * 256], F16, tag="ht", name="ht0")
            half = DT * 256 // 2
            nc.sync.dma_start(out=wk_sb[:, 0:16, :], in_=wk[:, 0:16, :])
            nc.sync.dma_start(out=ht0[:, 0:half], in_=ht[0][:, 0:half])
            nc.sync.dma_start(out=wk_sb[:, 16:32, :], in_=wk[:, 16:32, :])
            nc.sync.dma_start(out=ht0[:, half:], in_=ht[0][:, half:])
            ht_tiles[0] = ht0
            nc.sync.dma_start(out=wq_sb[:, 0:8, :], in_=wq[:, 0:8, :])
            nc.sync.dma_start(out=wq_sb[:, 8:16, :], in_=wq[:, 8:16, :])
            nc.sync.dma_start(out=wv_sb, in_=wv)
            ht_dma(1)
            nc.sync.dma_start(out=wq_sb[:, 16:24, :], in_=wq[:, 16:24, :])
            nc.sync.dma_start(out=wq_sb[:, 24:32, :], in_=wq[:, 24:32, :])
            nc.sync.dma_start(out=cos_sb, in_=cosh)
            nc.sync.dma_start(out=sin_sb, in_=sinh)
            ht_dma(2)
            nc.sync.dma_start(out=wo_sb, in_=wo)

            for sl in range(NSL):
                ssl = slice(sl * 512, (sl + 1) * 512)
                psQ = [psA.tile([128, 512], F32, tag="psA", name=f"psQ{g}")
                       for g in range(HQ)]
                psK = psA.tile([128, 512], F32, tag="psA", name="psK")
                psV = psA.tile([128, 512], F32, tag="psA", name="psV")
                def kpass(ht_t, hcols):
                    for d in range(DT):
                        nc.tensor.matmul(
                            psK[:, hcols], wk_sb[:, d, :],
                            ht_t[:, d * 256:(d + 1) * 256],
                            start=(d == 0), stop=(d == DT - 1))

                def qpass(ht_t, hcols):
                    for d in range(DT):
                        mv = ht_t[:, d * 256:(d + 1) * 256]
                        for g in range(HQ):
                            nc.tensor.matmul(
                                psQ[g][:, hcols],
                                wq_sb[:, d, g * DH:(g + 1) * DH], mv,
                                start=(d == 0), stop=(d == DT - 1))

                def vpass(ht_t, h):
                    # natural layout, hiddenT block stationary
                    for sb in range(2):
                        vc = slice((2 * h + sb) * 128, (2 * h + sb + 1) * 128)
                        for d in range(DT):
                            nc.tensor.matmul(
                                psV[:, vc],
                                ht_t[:, d * 256 + sb * 128:
                                     d * 256 + (sb + 1) * 128],
                                wv_sb[:, d, :],
                                start=(d == 0), stop=(d == DT - 1))

                def rope(raw, dstT):
                    pr = trA.tile([128, 512], F32, tag="pr", name="pr")
                    nc.tensor.matmul(pr, rot_sb, raw, start=True, stop=True)
                    prh = php.tile([128, 512], F16, tag="prh")
                    nc.scalar.copy(prh, pr)
                    rs = rtp.tile([128, 512], F16, tag="rs")
                    nc.vector.tensor_mul(rs, prh, sin_sb[:, ssl])
                    cc = rtp.tile([128, 512], F16, tag="cc")
                    nc.vector.tensor_mul(cc, raw, cos_sb[:, ssl])
                    nc.vector.tensor_add(dstT[:, ssl], cc, rs)

                def ht_get(h):
                    hs = 2 * sl + h
                    if hs + 2 < 2 * NSL and (hs + 2) not in ht_tiles:
                        ht_dma(hs + 2)
                    return ht_tiles.pop(hs)

                # h0: k first (its weights arrive first); h1: q first with
                # eager drains so the PSUM accumulators release while the
                # k/v passes still run on the PE (faster A->C transition)
                ht0 = ht_get(0)
                kpass(ht0, slice(0, 256))
                qpass(ht0, slice(0, 256))
                vpass(ht0, 0)
                ht1 = ht_get(1)
                qpass(ht1, slice(256, 512))
                qraws = []
                for g in range(HQ):
                    qr = rp.tile([128, 512], F16, tag="raw", name=f"qraw{g}")
                    nc.scalar.copy(qr, psQ[g])
                    qraws.append(qr)
                kpass(ht1, slice(256, 512))
                for g in range(HQ):
                    rope(qraws[g], qT[g])
                kr = rp.tile([128, 512], F16, tag="raw", name="kraw")
                nc.scalar.copy(kr, psK)
                vpass(ht1, 1)
                rope(kr, kT)
                nc.scalar.copy(v_all[:, 4 * sl:4 * sl + 4, :], psV)

        # ---------------- phase B+C: attention + o_proj ----------------
        outT_r = outT.rearrange("(x p) s -> p x s", p=128)
        with (
            tc.tile_pool(name="psO", bufs=2, space="PSUM") as psO,
            tc.tile_pool(name="trS", bufs=2, space="PSUM") as trS,
            tc.tile_pool(name="expp", bufs=6) as ep,
            tc.tile_pool(name="accp", bufs=4) as ap_,
            tc.tile_pool(name="dnp", bufs=4) as dp,
            tc.tile_pool(name="ocp", bufs=4) as ocp,
        ):
            def emit_pf_pair(qs, dpair):
                qsl = slice(qs * QSL, (qs + 1) * QSL)
                pf = trS.tile([128, 1024], F32, tag="trS", name="pf")
                for g in range(HQ):
                    for i in range(2):
                        nc.tensor.matmul(
                            pf[:, i * 512:(i + 1) * 512],
                            wo_sb[:, g, (2 * dpair + i) * 128:
                                  (2 * dpair + i + 1) * 128],
                            o_attn[g][:, qsl],
                            start=(g == 0), stop=(g == HQ - 1))
                oc = ocp.tile([128, 1024], F16, tag="oc")
                # parallel half-casts (ACT + DVE) release the PSUM slot fast
                nc.scalar.copy(oc[:, 0:512], pf[:, 0:512])
                nc.vector.tensor_copy(oc[:, 512:1024], pf[:, 512:1024])
                nc.sync.dma_start(
                    out=outT_r[:, 2 * dpair:2 * dpair + 2, qsl], in_=oc)

            pend_pv = None  # pv of the previous tile, emitted one tile late

            def emit_pv(pv):
                po, kind, exd, t, dd, nkt = pv
                for p in range(2):
                    for i in range(2):
                        h = slice(i * 512 + dd, (i + 1) * 512)
                        if kind == "pair":
                            mv = exd[p][:, h]
                        else:  # packed: 4 heads side by side, width w each
                            ex, w = exd
                            g = 2 * p + i
                            mv = ex[:, g * w:(g + 1) * w]
                        nc.tensor.matmul(po[p][:, h], v_all[:, t, :], mv,
                                         start=(t == 0),
                                         stop=(t == nkt - 1))

            for qs in range(NQS):
                qsl = slice(qs * QSL, (qs + 1) * QSL)
                nkt = 4 * (qs + 1)
                po = [psO.tile([128, 1024], F32, tag="po", name=f"po{p}")
                      for p in range(2)]
                acc = [ap_.tile([128, 1024], F16, tag="acc", name=f"acc{p}")
                       for p in range(2)]
                # o_proj filler work from the previous q-slice, spread over
                # t=1..nkt-1 (fillers at t=0 would wait on the fresh drain)
                pending = list(range(NDP)) if qs > 0 else []
                reserve = 3 if pending else 0
                navail = len(pending) - reserve
                for t in range(nkt):
                    n_fill = 0
                    if pending and t > 0:
                        n_fill = (t * navail) // (nkt - 1) \
                            - ((t - 1) * navail) // (nkt - 1)
                    delta = t * 128 - qs * QSL
                    dd = max(delta, 0)
                    ktile = kT[:, t * 128:(t + 1) * 128]
                    qslc = slice(qs * QSL + dd, (qs + 1) * QSL)
                    if dd >= 256:
                        # narrow diagonal tile: pack all 4 heads' trimmed
                        # scores in ONE slot (halves ring pressure, one exp)
                        w = 512 - dd
                        ps = trS.tile([128, 1024], F32, tag="trS", name="psD")
                        for g in range(HQ):
                            nc.tensor.matmul(ps[:, g * w:(g + 1) * w],
                                             ktile, qT[g][:, qslc],
                                             start=True, stop=True)
                        for g in range(HQ):
                            blk = slice(g * w, g * w + 128)
                            nc.vector.tensor_add(ps[:, blk], ps[:, blk],
                                                 mneg_sb)
                        ex = ep.tile([128, 1024], F16, tag="ex")
                        nc.scalar.activation(ex[:, 0:4 * w], ps[:, 0:4 * w],
                                             Exp, bias=cb_sb, scale=scale)
                        this_pv = (po, "packed", (ex, w), t, dd, nkt)
                    else:
                        exs = []
                        for p in range(2):
                            ps = trS.tile([128, 1024], F32, tag="trS",
                                          name="psS")
                            for i in range(2):
                                g = 2 * p + i
                                nc.tensor.matmul(
                                    ps[:, i * 512 + dd:(i + 1) * 512],
                                    ktile, qT[g][:, qslc],
                                    start=True, stop=True)
                            if delta >= 0:
                                for i in range(2):
                                    blk = slice(i * 512 + dd,
                                                i * 512 + dd + 128)
                                    nc.vector.tensor_add(ps[:, blk],
                                                         ps[:, blk], mneg_sb)
                            ex = ep.tile([128, 1024], F16, tag="ex")
                            # dd=128: full-pair exp is as cheap as two trimmed
                            # ops; the stale strips are never read (pv and the
                            # denominator adds are trimmed to [dd:512])
                            nc.scalar.activation(ex, ps, Exp,
                                                 bias=cb_sb, scale=scale)
                            exs.append(ex)
                        this_pv = (po, "pair", exs, t, dd, nkt)
                    # previous tile's pv: independent PE work that covers the
                    # exp latency and the score-slot WAR on the in-order PE
                    if pend_pv is not None:
                        emit_pv(pend_pv)
                    for _ in range(n_fill):
                        if len(pending) > reserve:
                            emit_pf_pair(qs - 1, pending.pop(0))
                    pend_pv = this_pv
                    # ping-pong accumulate (out != in0 keeps DVE fast modes)
                    for p in range(2):
                        if t == 0:
                            nc.vector.tensor_copy(acc[p], exs[p])
                        elif dd == 0:
                            na = ap_.tile([128, 1024], F16, tag="acc",
                                          name=f"acc{p}")
                            nc.vector.tensor_add(na, acc[p], exs[p])
                            acc[p] = na
                        elif dd >= 256:
                            for i in range(2):
                                h = slice(i * 512 + dd, (i + 1) * 512)
                                g = 2 * p + i
                                nc.vector.tensor_add(acc[p][:, h],
                                                     acc[p][:, h],
                                                     ex[:, g * w:(g + 1) * w])
                        else:
                            for i in range(2):
                                h = slice(i * 512 + dd, (i + 1) * 512)
                                nc.vector.tensor_add(acc[p][:, h],
                                                     acc[p][:, h],
                                                     exs[p][:, h])

                # denominators (need only the DVE acc chain, not the pvs)
                dns = []
                pds = []
                for p in range(2):
                    pd = trS.tile([128, 1024], F32, tag="trS", name="pd")
                    for i in range(2):
                        h = slice(i * 512, (i + 1) * 512)
                        nc.tensor.matmul(pd[:, h], ones_sb, acc[p][:, h],
                                         start=True, stop=True)
                    pds.append(pd)
                # last tile's pv before the reciprocal chain needs it
                emit_pv(pend_pv)
                pend_pv = None
                pdcs = []
                for p in range(2):
                    # ACT copy releases the PSUM slot and runs parallel to
                    # the DVE reciprocal/normalize chain
                    pdc = dp.tile([128, 1024], F32, tag="pdc")
                    nc.scalar.copy(pdc, pds[p])
                    pdcs.append(pdc)
                for p in range(2):
                    dn = dp.tile([128, 1024], F32, tag="dn")
                    nc.vector.reciprocal_approx_fast(out=dn, in_=pdcs[p])
                    for i in range(2):
                        g = 2 * p + i
                        h = slice(i * 512, (i + 1) * 512)
                        nc.vector.tensor_mul(o_attn[g][:, qsl],
                                             po[p][:, h], dn[:, h])
                # reserved PE filler covers the reciprocal+drain latency
                while pending:
                    emit_pf_pair(qs - 1, pending.pop(0))

            for dpair in range(NDP):
                emit_pf_pair(NQS - 1, dpair)

    nc.compile()
    return nc


def make_tables(cfg: Cfg, position_ids: np.ndarray):
    """cos/sin [128, S] fp16: row d holds cos/sin(pos * invfreq[d % 64])."""
    half = cfg.DH // 2
    inv = 1.0 / (cfg.theta ** (np.arange(half, dtype=np.float64) * 2.0 / cfg.DH))
    pos = np.asarray(position_ids).reshape(-1).astype(np.float64)
    ang = inv[:, None] * pos[None, :]
    cosT = np.concatenate([np.cos(ang), np.cos(ang)], 0).astype(np.float16)
    sinT = np.concatenate([np.sin(ang), np.sin(ang)], 0).astype(np.float16)
    return cosT, sinT


def make_rotm(cfg: Cfg):
    """[128, 256] fp16: cols 0-127 rotate-half matrix (out = M^T @ x ->
    out[:64] = -x[64:], out[64:] = x[:64]); cols 128-255 all ones."""
    half = cfg.DH // 2
    m = np.zeros((128, 256), np.float16)
    for i in range(half):
        m[i + half, i] = -1.0
        m[i, i + half] = 1.0
    m[:, 128:] = 1.0
    return m


def make_mneg():
    """[128, 128] f32 strict lower-triangular NEG: mask[p, j] = NEG if j < p."""
    j = np.arange(128)[None, :]
    p = np.arange(128)[:, None]
    return np.where(j < p, NEG, 0.0).astype(np.float32)


_cache = threading.Lock()
_nc_full = None


def _get_nc():
    global _nc_full
    with _cache:
        if _nc_full is None:
            _nc_full = build_nc(FULL)
    return _nc_full


def _prep_shared(cfg: Cfg, position_ids, hidden_states):
    S, D, DT = cfg.S, cfg.D, cfg.D // 128
    h16 = np.asarray(hidden_states, np.float32).reshape(S, D).T.astype(np.float16)
    # [D, S] -> [2*NSL half-slices, 128 partitions, DT*256]
    ht = np.ascontiguousarray(
        h16.reshape(DT, 128, 2 * (S // 512), 256).transpose(2, 1, 0, 3)
        .reshape(2 * (S // 512), 128, DT * 256))
    cosT, sinT = make_tables(cfg, position_ids)
    return ht, cosT, sinT


def core_inputs(cfg: Cfg, c: int, shared, Wq, Wk, Wv, Wo):
    S, D, HQ, DH = cfg.S, cfg.D, cfg.HQ, cfg.DH
    DT = D // 128
    ht, cosT, sinT = shared
    qc = slice(c * HQ * DH, (c + 1) * HQ * DH)
    kc = slice(c * DH, (c + 1) * DH)

    def wtile(w, cols, n):
        w16 = np.asarray(w, np.float32)[:, cols].astype(np.float16)
        return np.ascontiguousarray(w16.reshape(DT, 128, n).transpose(1, 0, 2))

    wo16 = np.asarray(Wo, np.float32)[qc, :].astype(np.float16)
    wo_t = np.ascontiguousarray(wo16.reshape(HQ, 128, D).transpose(1, 0, 2))
    return {
        "ht": ht,
        "wq": wtile(Wq, qc, HQ * DH),
        "wk": wtile(Wk, kc, DH),
        "wv": wtile(Wv, kc, DH),
        "wo": wo_t,
        "cosh": cosT,
        "sinh": sinT,
        "mneg": make_mneg(),
        "rotm": make_rotm(cfg),
    }


def kernel(position_ids, hidden_states, Wq, Wk, Wv, Wo, _trace=False):
    from concourse.bass_utils import run_bass_kernel_spmd

    cfg = FULL
    nc = _get_nc()
    shared = _prep_shared(cfg, position_ids, hidden_states)
    in_maps = [core_inputs(cfg, c, shared, Wq, Wk, Wv, Wo)
               for c in range(cfg.cores)]
    res = run_bass_kernel_spmd(nc, in_maps, core_ids=list(range(cfg.cores)),
                               trace=_trace)
    out = np.zeros((cfg.S, cfg.D), np.float64)
    for c in range(cfg.cores):
        out += res.results[c]["outT"].T.astype(np.float64)
    ret = out.astype(np.float32).reshape(1, cfg.S, cfg.D)
    if _trace:
        return ret, res
    return ret
